# revision 18
# baseline (speedup 1.0000x reference)
"""Trainium2 Bass kernel for ConvTranspose4d (T: 3-tap valid conv; D/H/W:
stride-2 k=3 p=1 transposed conv). Self-contained: hardcoded shapes.

x: [1, 8, 8, 24, 48, 48] f32, weight: [8, 8, 3, 3, 3, 3] f32
out: [1, 8, 6, 47, 95, 95] f32

Strategy (8 NeuronCores, data-parallel over D):
  - Core j computes output od = 6j..6j+5 (core 7 drops od 47); needs input
    slices id0..id0+3 (id0 = min(3j, 20)).
  - Temporal 3-tap conv and D-axis stride-2 transposed conv fold into the
    matmul stationary operand as a banded weight matrix:
      lhsT[K=128=(slot4, cin8, id4), M=(fbit2 x 48=(cout8*6+od))]
    where slot = kt + fbit holds temporal plane 2i+slot of frame-pair i.
  - All I/O is bf16 (host converts): input planes pre-laid-out per
    frame-pair as [3, 128, 49, 49] (zero-padded), triple-buffered in SBUF
    so the PE never stalls on WAR hazards; bands are [128, 9, 128].
  - H/W parities are 4 output classes (ph, pw); each accumulates 1/2/2/4
    shifted-view taps in PSUM (kh = ph - 2*dh + 1).
  - Staging is PARITY-PLANAR: each class region is contiguous per
    partition, so PSUM->SBUF copies (DVE for ph=0, ACT for ph=1) are
    contiguous; the host interleaves the stride-2 H/W grids during the
    gather. Output DMA (bf16) is split across both HWDGE queues.
"""
import numpy as np

COMPUTE = "bfloat16"

TAPS = {
    (0, 0): [(0, 0)],
    (0, 1): [(0, 0), (0, 1)],
    (1, 0): [(0, 0), (1, 0)],
    (1, 1): [(0, 0), (0, 1), (1, 0), (1, 1)],
}
TAP_LIST = [(ph, pw, dh, dw) for (ph, pw), tl in TAPS.items() for (dh, dw) in tl]
CHUNK_START = [0, 10, 20, 30, 40]
CHUNK_N = [10, 10, 10, 10, 8]
PAIRS = [(0, 1), (2, 3), (4,)]
# class -> (region offset in stg, rows, cols)
REGION = {
    (0, 0): (0, 48, 48),
    (0, 1): (2304, 48, 47),
    (1, 0): (4560, 47, 48),
    (1, 1): (6816, 47, 47),
}

_CACHE = {}


def _bf16():
    import ml_dtypes
    return ml_dtypes.bfloat16


def _build_bands(W, j):
    """W: [cin8, cout8, kt3, kd3, kh3, kw3] -> [128, 9, 128] f32.
    K row = slot*32 + cin*4 + id (slot = kt + fbit);
    M col = fbit*48 + cout*6 + od (od 0..5; cols 96..127 zero)."""
    id0 = min(3 * j, 20)
    B = np.zeros((128, 9, 128), np.float32)
    ci = np.arange(8)
    co = np.arange(8)
    for t, (ph, pw, dh, dw) in enumerate(TAP_LIST):
        kh = ph - 2 * dh + 1
        kw = pw - 2 * dw + 1
        for fbit in range(2):
            for kt in range(3):
                slot = kt + fbit
                for idl in range(4):
                    for od in range(6):
                        od_g = 6 * j + od
                        if od_g > 46:
                            continue
                        kd = od_g - 2 * (id0 + idl) + 1
                        if not (0 <= kd <= 2):
                            continue
                        krow = slot * 32 + ci * 4 + idl
                        mcol = fbit * 48 + co * 6 + od
                        B[krow[:, None], t, mcol[None, :]] = W[:, :, kt, kd, kh, kw]
    return B


def _free_view(base, off, dims):
    """Hand-built AP: keep base's partition dim, replace free dims with
    [(step, count), ...] (element units) at extra offset `off`."""
    a = base.copy()
    v = a.ap
    part = v.to_list()[0]
    v.clear()
    v.append(part)
    for sc in dims:
        v.append(list(sc))
    a.ap = v
    a.offset = a.offset + off
    return a


def _build_program():
    import concourse.bacc as bacc
    import concourse.tile as tile
    from concourse import mybir

    f32 = mybir.dt.float32
    bf16 = mybir.dt.bfloat16

    nc = bacc.Bacc("TRN2", target_bir_lowering=False, debug=False)
    # flat per-partition layout [bands(1152) | p0(2401) | p1(2401) | p2(2401)]
    # -> one SBUF tile, big DMA descriptors, range-split loads so the data
    # matmul #0 needs (bands + plane-0 top rows) lands first.
    xs_ap = nc.dram_tensor("xs", [128, 8355], bf16, kind="ExternalInput").ap()
    # [pair, fbit, co*6+od, pos] -- matches stg partition order so each pair
    # drains as ONE big DMA (96 partitions x 18 KB).
    out_ap = nc.dram_tensor("out", [3, 2, 48, 9025], bf16, kind="ExternalOutput").ap()

    with tile.TileContext(nc, trace_sim=False) as tc:
        with (
            tc.tile_pool(name="bp", bufs=1) as bp,
            tc.tile_pool(name="sp", bufs=3) as sp,
            tc.tile_pool(name="ps", bufs=8, space="PSUM") as ps,
        ):
            # PE warm-up: dummy matmuls sized to end right as plane 0 lands,
            # so the HAM clock-gate (K=8/8) is open and STAYS open (no >3.4us
            # PE-idle gap) when the real stream starts.
            dz = bp.tile([128, 64], bf16)
            nc.gpsimd.memset(dz[:], 0.0)
            wps = ps.tile([128, 512], f32, name="warm", tag="ps")
            for _ in range(38):
                nc.tensor.matmul(wps[0:64, 0:64], dz[:], dz[:], start=True,
                                 stop=True)

            xa = bp.tile([128, 8355], bf16)
            # A1 = bands + plane-0 rows 0-24 (feeds chunks 0-1), A2 = rest of
            # plane 0.  A full 128-partition DMA engages all 16 SDMA engines,
            # so both go on the sync HWDGE queue back to back.  B = planes
            # 1-2 on the gpsimd SWDGE queue (needed ~10us later).
            A1, A2 = 2377, 3553
            nc.sync.dma_start(out=xa[:, 0:A1], in_=xs_ap[:, 0:A1])
            nc.sync.dma_start(out=xa[:, A1:A2], in_=xs_ap[:, A1:A2])
            nc.gpsimd.dma_start(out=xa[:, A2:8355], in_=xs_ap[:, A2:8355])

            for i in range(3):
                poff = 1152 + i * 2401
                stg = sp.tile([128, 9025], bf16, name=f"stg{i}", tag="stg")
                corder = list(TAPS) if i < 2 else list(TAPS)[::-1]
                for (ph, pw) in corder:
                    taps = TAPS[(ph, pw)]
                    roff, _, nmw = REGION[(ph, pw)]
                    for c in range(5):
                        pt = ps.tile([128, 512], f32, name="ps", tag="ps")
                        mh0, nmh = CHUNK_START[c], CHUNK_N[c]
                        for ti, (dh, dw) in enumerate(taps):
                            t_idx = TAP_LIST.index((ph, pw, dh, dw))
                            lhsT = xa[:, t_idx * 128:(t_idx + 1) * 128]
                            rhs = _free_view(
                                xa[:], poff + (mh0 + dh) * 49 + dw,
                                [(49, nmh), (1, 48)])
                            nc.tensor.matmul(
                                pt[:, 0:nmh * 48], lhsT, rhs,
                                start=(ti == 0), stop=(ti == len(taps) - 1),
                            )
                        # contiguous copy PSUM -> class-planar staging.  For
                        # the final pair, alternate DVE/ACT per chunk so the
                        # copies chase the matmuls instead of queueing on one
                        # engine after the stream ends.
                        nmh_c = nmh if c < 4 else CHUNK_N[4] - ph
                        src = _free_view(pt[0:96], 0, [(48, nmh_c), (1, nmw)])
                        doff = roff + CHUNK_START[c] * nmw
                        dst = _free_view(stg[0:96], doff,
                                         [(nmw, nmh_c), (1, nmw)])
                        use_dve = (c % 2 == 0) if i == 2 else (ph == 0)
                        if use_dve:
                            nc.vector.tensor_copy(dst, src)
                        else:
                            nc.scalar.copy(dst, src)
                # output DMA: one big transfer per pair (96 partitions x
                # 18 KB contiguous rows), overlapped with the next pair's
                # matmul stream.  The last pair (reversed class order: ph=1
                # staged first) drains in two ph-halves on the two HWDGE
                # queues so the tail is just the ph=0 half.
                if i < 2:
                    eng = nc.scalar if i == 0 else nc.sync
                    eng.dma_start(out=out_ap[i],
                                  in_=_free_view(stg[0:96], 0, [(1, 9025)]))
                else:
                    # final drain per class, in staging-completion order, so
                    # the tail is only the last ~430 KB class; HWDGE only
                    dr = [(6816, 2209, nc.scalar), (4560, 2256, nc.sync),
                          (2304, 2256, nc.scalar), (0, 2304, nc.sync)]
                    for r0, sz, eng in dr:
                        eng.dma_start(
                            out=out_ap[2, :, :, r0:r0 + sz],
                            in_=_free_view(stg[0:96], r0, [(1, sz)]))

    nc.compile()
    return nc


def _get_program():
    if "nc" not in _CACHE:
        _CACHE["nc"] = _build_program()
    return _CACHE["nc"]


def run(x, weight, trace=False):
    from concourse.bass_utils import run_bass_kernel_spmd

    bf16 = _bf16()
    x = np.asarray(x, dtype=np.float32)
    weight = np.asarray(weight, dtype=np.float32)
    in_maps = []
    for j in range(8):
        id0 = min(3 * j, 20)
        xs = np.zeros((3, 4, 8, 4, 49, 49), np.float32)
        for i in range(3):
            for slot in range(4):
                # [c, id, 48, 48]
                xs[i, slot, :, :, :48, :48] = x[0, :, 2 * i + slot, id0:id0 + 4]
        # partition = slot*32 + c*4 + idl  ->  order [i, slot, c, idl, h, w]
        xs = xs.reshape(3, 128, 2401).transpose(1, 0, 2).reshape(128, 7203)
        bands = _build_bands(weight, j).reshape(128, 1152)
        in_maps.append({
            "xs": np.concatenate([bands, xs], axis=1).astype(bf16),
        })
    nc = _get_program()
    res = run_bass_kernel_spmd(nc, in_maps, core_ids=list(range(8)), trace=trace)
    full = np.zeros((1, 8, 6, 47, 95, 95), np.float32)
    for j in range(8):
        nod = min(6, 47 - 6 * j)
        oj = np.asarray(res.results[j]["out"]).astype(np.float32)
        # [pair, fbit, co*6+od, pos] -> [co, frame, od, pos]
        oj = oj.reshape(3, 2, 8, 6, 9025).transpose(2, 0, 1, 3, 4).reshape(
            8, 6, 6, 9025)
        oj = oj[:, :, :nod]  # [8, 6, nod, 9025]
        dst = full[0, :, :, 6 * j:6 * j + nod]
        for (ph, pw), (roff, nr, ncol) in REGION.items():
            dst[..., ph::2, pw::2] = oj[..., roff:roff + nr * ncol].reshape(
                8, 6, nod, nr, ncol)
    return full, res


def kernel(x, weight):
    return run(x, weight)[0]

